# revision 39
# baseline (speedup 1.0000x reference)
"""Trainium2 Bass kernel for nn_Nequix (e3nn-style message-passing layer).

Sharding: edges partitioned by receiver range across 8 cores (500 nodes each);
node features and weights replicated; no collectives. Within a core, edges are
grouped into 4 windows of 128 receiver nodes; scatter-sum is one-hot matmuls
accumulating in PSUM per window.

v2 layout (all per core):
  xgT  [C, TT, 4, 128]  pre-gathered sender features (host gather), per-tile
                        lhsT for m = x[send] @ W1 on the PE (no device gather)
  OHW  [128, TT, 4, WIN] one-hot + Y1-weighted one-hots (host); scatter lhsT
  rb2  [16, EPAD/2]     radial basis, 2 edge-tiles packed per column pair-slot
  radial MLP            block-diagonal 2-slot weights -> h slabs [128, EW/2]
  L3                    per-tile h3 lhsT -> w [128e, 4C] PSUM, ACT evac
  products (DVE)        9 slab mults m (.) w per 8-tile chunk
  scatter               agg PSUM [128, 2, 512]: A=[s0|v1x|v1y|v1z],
                        B=[v0x|s1|v0y|v0z]; lhsT = OHW slices
  final                 transpose agg, linear_2 + species skip, silu gates;
                        outT [512, 512] f32
"""
import math
import os
import numpy as np

KSTAGE = int(os.environ.get("KSTAGE", "9"))
KDEBUG = int(os.environ.get("KDEBUG", "0"))

N, E, C, NS, RB, H = 4000, 128000, 128, 5, 8, 64
AVG_N = 32.0
NCORES = 8
NV = 500
WIN = 128
NWIN = 4
SQ3 = math.sqrt(3.0)


def _prep_host(inputs):
    import ml_dtypes
    bf = ml_dtypes.bfloat16
    f32 = np.float32

    xs = np.asarray(inputs["x_scalars"], f32)
    xv = np.asarray(inputs["x_vectors"], f32)
    ev = np.asarray(inputs["edge_vectors"], f32)
    rb = np.asarray(inputs["radial_basis"], f32)
    W1_0 = np.asarray(inputs["W1_0"], f32)
    W1_1 = np.asarray(inputs["W1_1"], f32)
    w0 = np.asarray(inputs["rmlp_w0"], f32)
    w1 = np.asarray(inputs["rmlp_w1"], f32)
    w2 = np.asarray(inputs["rmlp_w2"], f32)
    w3 = np.asarray(inputs["rmlp_w3"], f32).copy()
    W2_0 = np.asarray(inputs["W2_0"], f32)
    W2_1 = np.asarray(inputs["W2_1"], f32)
    Wsk0 = np.asarray(inputs["Wsk0"], f32)
    Wsk1 = np.asarray(inputs["Wsk1"], f32)
    species = np.asarray(inputs["species"]).astype(np.int64)
    send = np.asarray(inputs["senders"]).astype(np.int64)
    recv = np.asarray(inputs["receivers"]).astype(np.int64)

    inv_c = f32(1.0 / math.sqrt(C))
    W1_0f = W1_0 * inv_c
    W1_1f = W1_1 * inv_c
    w3f = w3 * f32(1.0 / math.sqrt(AVG_N))
    w3f[:, C:2 * C] *= f32(1.0 / SQ3)
    inv_2c = f32(1.0 / math.sqrt(2 * C))
    W2_0f = W2_0 * inv_2c
    W2_1f = W2_1 * inv_2c
    Wsk0f = Wsk0 * inv_c          # [NS, C, 2C]
    Wsk1f = Wsk1 * inv_c          # [NS, C, C]

    # edge geometry (host): Y1 = sqrt(3) * unit(edge_vectors)
    r = np.sqrt((ev * ev).sum(1, keepdims=True))
    Y1 = SQ3 * ev / np.maximum(r, 1e-12)                               # [E,3]

    # ---- edge shard assignment: core by receiver range, window by rloc//WIN
    core_of = recv // NV
    rloc_all = recv - core_of * NV
    win_of = rloc_all // WIN
    per_core_win = []
    maxcnt = 0
    for i in range(NCORES):
        wins = []
        for w in range(NWIN):
            pw = np.nonzero((core_of == i) & (win_of == w))[0]
            wins.append(pw)
            maxcnt = max(maxcnt, len(pw))
        per_core_win.append(wins)
    EW = ((maxcnt + 255) // 256) * 256     # even tile count per window
    TW = EW // 128
    EPAD = EW * NWIN
    TT = EPAD // 128

    # ---- shared constants
    # block-diagonal 2-slot radial weights
    w0b = np.zeros((16, 128), f32)
    w0b[:RB, :H] = w0
    w0b[RB:2 * RB, H:2 * H] = w0
    w1b = np.zeros((128, 128), f32)
    w1b[:H, :H] = w1
    w1b[H:, H:] = w1
    w2b = np.zeros((128, 128), f32)
    w2b[:H, :H] = w2
    w2b[H:, H:] = w2
    w3d = np.concatenate([w3f, w3f], axis=0)                           # [128,4C]

    W20L = np.stack([W2_0f[0:128, :], W2_0f[128:256, :]], axis=1)      # [128,2,256]
    W21L = np.stack([W2_1f[0:128, :], W2_1f[128:256, :]], axis=1)      # [128,2,128]
    Wsk0L = Wsk0f.transpose(1, 0, 2)                                   # [128,NS,256]
    Wsk1L = Wsk1f.transpose(1, 0, 2)                                   # [128,NS,128]

    NPAD = 4096
    xsT = np.zeros((C, NPAD), f32)
    xsT[:, :N] = xs.T
    xvT = np.zeros((3, C, NPAD), f32)
    for i in range(3):
        xvT[i, :, :N] = xv[:, :, i].T

    consts = dict(
        xsT=xsT.astype(bf), xvT=xvT.astype(bf),
        W10=W1_0f.astype(bf), W11=W1_1f.astype(bf),
        w0b=w0b.astype(bf), w1b=w1b.astype(bf), w2b=w2b.astype(bf),
        w3d=w3d.astype(bf),
        W20=W20L.astype(bf), W21=W21L.astype(bf),
        Wsk0=Wsk0L.astype(bf), Wsk1=Wsk1L.astype(bf),
    )

    # ---- per-core tensors
    cores = []
    for i in range(NCORES):
        send_p = np.zeros(EPAD, np.int64)
        rloc_p = np.zeros(EPAD, np.int64)
        val_p = np.zeros(EPAD, f32)
        rb_p = np.zeros((EPAD, RB), f32)
        Y1_p = np.zeros((EPAD, 3), f32)
        for w in range(NWIN):
            pw = per_core_win[i][w]
            k = len(pw)
            sl = slice(w * EW, w * EW + k)
            send_p[sl] = send[pw]
            rloc_p[sl] = rloc_all[pw] - w * WIN
            val_p[sl] = 1.0
            rb_p[sl] = rb[pw]
            Y1_p[sl] = Y1[pw]

        # gather indices, int16 (pad slots gather row 0; one-hot row is zero)
        idx16 = send_p.astype(np.int16).reshape(EPAD // 16, 16).T
        sendidx = np.tile(idx16, (8, 1))                               # [128, EPAD//16]

        # OHW[p, t, 0, n] = onehot; [p, t, 1+i, n] = onehot * Y1_i
        ohw = np.zeros((EPAD, 4, WIN), f32)
        ar = np.arange(EPAD)
        ohw[ar, 0, rloc_p] = val_p
        for j in range(3):
            ohw[ar, 1 + j, rloc_p] = val_p * Y1_p[:, j]
        OHW = ohw.reshape(TT, 128, 4, WIN).transpose(1, 0, 2, 3)

        # rb2[s*8+r, w*(EW/2) + P*128 + p] = rb[edge (w, (2P+s)*128+p), r]
        arr = rb_p.reshape(NWIN, TW // 2, 2, 128, RB)
        rb2 = arr.transpose(2, 4, 0, 1, 3).reshape(16, EPAD // 2)

        sl_n = slice(i * NV, (i + 1) * NV)
        xs_my = np.zeros((512, C), f32)
        xs_my[:NV] = xs[sl_n]
        xv_my = np.zeros((512, C, 3), f32)
        xv_my[:NV] = xv[sl_n]
        soh = np.zeros((512, NS), f32)
        soh[np.arange(NV), species[sl_n]] = 1.0
        xskT = np.einsum("nc,nk->ckn", xs_my, soh)                     # [128,NS,512]
        xvkT = np.einsum("nci,nk->ckin", xv_my, soh)                   # [128,NS,3,512]

        cores.append(dict(
            sendidx=np.ascontiguousarray(sendidx),
            OHW=np.ascontiguousarray(OHW.astype(bf)),
            rb2=np.ascontiguousarray(rb2.astype(bf)),
            xskT=np.ascontiguousarray(xskT.astype(bf)),
            xvkT=np.ascontiguousarray(xvkT.astype(bf)),
        ))
    return consts, cores, EW


def _build_program(EW):
    import concourse.bacc as bacc
    import concourse.mybir as mybir
    import concourse.tile as tile
    from concourse.masks import make_identity

    f32 = mybir.dt.float32
    bf = mybir.dt.bfloat16
    AF = mybir.ActivationFunctionType
    OP = mybir.AluOpType

    TW = EW // 128
    EPAD = EW * NWIN
    TT = EPAD // 128
    CH = 8                                  # edge tiles per compute chunk

    nc = bacc.Bacc("TRN2", target_bir_lowering=False)

    def param(name, shape, dtype):
        return nc.declare_dram_parameter(name, list(shape), dtype, isOutput=False)

    NPAD = 4096
    i16 = mybir.dt.int16
    xsT_d = param("xsT", (C, NPAD), bf)
    xvT_d = param("xvT", (3, C, NPAD), bf)
    sendidx_d = param("sendidx", (128, EPAD // 16), i16)
    W10_d = param("W10", (C, C), bf)
    W11_d = param("W11", (C, C), bf)
    w0b_d = param("w0b", (16, 128), bf)
    w1b_d = param("w1b", (128, 128), bf)
    w2b_d = param("w2b", (128, 128), bf)
    w3d_d = param("w3d", (128, 4 * C), bf)
    W20_d = param("W20", (C, 2, 2 * C), bf)
    W21_d = param("W21", (C, 2, C), bf)
    Wsk0_d = param("Wsk0", (C, NS, 2 * C), bf)
    Wsk1_d = param("Wsk1", (C, NS, C), bf)
    OHW_d = param("OHW", (128, TT, 4, WIN), bf)
    rb2_d = param("rb2", (16, EPAD // 2), bf)
    xskT_d = param("xskT", (C, NS, 512), bf)
    xvkT_d = param("xvkT", (C, NS, 3, 512), bf)
    outT_d = nc.declare_dram_parameter("outT", [4 * C, 512], f32, isOutput=True)
    if KDEBUG:
        dbgm_d = nc.declare_dram_parameter("dbgm", [128, 8, 512], bf, isOutput=True)
        dbgw_d = nc.declare_dram_parameter("dbgw", [128, 8, 512], bf, isOutput=True)
        dbg1_d = nc.declare_dram_parameter("dbg1", [128, 8, 512], bf, isOutput=True)
        dbg2_d = nc.declare_dram_parameter("dbg2", [128, 8, 384], bf, isOutput=True)
        dbg3_d = nc.declare_dram_parameter("dbg3", [128, 8, 256], bf, isOutput=True)
        dbga_d = nc.declare_dram_parameter("dbga", [128, 3, 512], bf, isOutput=True)

    with tile.TileContext(nc) as tc:
        with (
            tc.tile_pool(name="dram", bufs=1, space="DRAM") as dpool,
            tc.tile_pool(name="const", bufs=1) as cpool,
            tc.tile_pool(name="xload", bufs=3) as xpool,
            tc.tile_pool(name="rbload", bufs=2) as rbpool,
            tc.tile_pool(name="hslab", bufs=2) as hpool,
            tc.tile_pool(name="eload", bufs=3) as epool,
            tc.tile_pool(name="mw", bufs=2) as mwpool,
            tc.tile_pool(name="prod", bufs=2) as ppool,
            tc.tile_pool(name="fin", bufs=2) as fpool,
            tc.tile_pool(name="ps_w", bufs=3, space="PSUM") as ps_w,
            tc.tile_pool(name="ps_sm", bufs=1, space="PSUM") as ps_sm,
            tc.tile_pool(name="ps_agg", bufs=1, space="PSUM") as ps_agg,
        ):
            y_d = dpool.tile([NPAD, 4 * C], bf)
            def cload(dram, shape, dtype):
                t = cpool.tile(list(shape), dtype, tag=dram.name)
                nc.sync.dma_start(t[:], dram[:])
                return t

            W10_s = cload(W10_d, (C, C), bf)
            W11_s = cload(W11_d, (C, C), bf)
            w0b_s = cload(w0b_d, (16, 128), bf)
            w1b_s = cload(w1b_d, (128, 128), bf)
            w2b_s = cload(w2b_d, (128, 128), bf)
            w3d_s = cload(w3d_d, (128, 4 * C), bf)
            W20_s = cload(W20_d, (C, 2, 2 * C), bf)
            W21_s = cload(W21_d, (C, 2, C), bf)
            Wsk0_s = cload(Wsk0_d, (C, NS, 2 * C), bf)
            Wsk1_s = cload(Wsk1_d, (C, NS, C), bf)
            xskT_s = cload(xskT_d, (C, NS, 512), bf)
            xvkT_s = cload(xvkT_d, (C, NS, 3, 512), bf)
            sendidx_s = cload(sendidx_d, (128, EPAD // 16), i16)
            ident_s = cpool.tile([128, 128], bf)
            make_identity(nc, ident_s[:])

            # ================= phase Y: y = x @ W1 (all nodes) =================
            for nch in range(NPAD // 128):
                xs_t = xpool.tile([C, 128], bf, tag="xs")
                nc.sync.dma_start(xs_t[:], xsT_d[:, nch * 128:(nch + 1) * 128])
                psy = ps_w.tile([128, 4 * C], f32, tag="w")
                nc.tensor.matmul(psy[:, 0:C], lhsT=xs_t[:], rhs=W10_s[:],
                                 start=True, stop=True)
                for i in range(3):
                    xv_t = xpool.tile([C, 128], bf, tag="xv")
                    nc.sync.dma_start(xv_t[:], xvT_d[i, :, nch * 128:(nch + 1) * 128])
                    nc.tensor.matmul(psy[:, (1 + i) * C:(2 + i) * C], lhsT=xv_t[:],
                                     rhs=W11_s[:], start=True, stop=True)
                yb = xpool.tile([128, 4 * C], bf, tag="yb")
                nc.scalar.copy(yb[:], psy[:])
                nc.sync.dma_start(y_d[nch * 128:(nch + 1) * 128, :], yb[:])

            for w in range(NWIN if KSTAGE >= 1 else 0):
                # ---------------- radial MLP (2-slot packed) ----------------
                ncol = EW // 2
                rb2_t = rbpool.tile([16, ncol], bf, tag="rb2")
                nc.sync.dma_start(rb2_t[:], rb2_d[:, w * ncol:(w + 1) * ncol])
                h3 = hpool.tile([128, ncol], bf, tag="h3")
                h1 = hpool.tile([128, 512], bf, tag="h1")
                h2 = hpool.tile([128, 512], bf, tag="h2")

                def radial_chunk(c0):
                    c1 = min(c0 + 512, ncol)
                    nn = c1 - c0
                    ps0 = ps_sm.tile([128, 512], f32, tag="sm", name="ps0")
                    nc.tensor.matmul(ps0[:, :nn], lhsT=w0b_s[:],
                                     rhs=rb2_t[:, c0:c1], start=True, stop=True)
                    nc.scalar.activation(h1[:, :nn], ps0[:, :nn], AF.Silu)
                    ps1 = ps_sm.tile([128, 512], f32, tag="sm", name="ps1")
                    nc.tensor.matmul(ps1[:, :nn], lhsT=w1b_s[:], rhs=h1[:, :nn],
                                     start=True, stop=True)
                    nc.scalar.activation(h2[:, :nn], ps1[:, :nn], AF.Silu)
                    ps2 = ps_sm.tile([128, 512], f32, tag="sm", name="ps2")
                    nc.tensor.matmul(ps2[:, :nn], lhsT=w2b_s[:], rhs=h2[:, :nn],
                                     start=True, stop=True)
                    nc.scalar.activation(h3[:, c0:c1], ps2[:, :nn], AF.Silu)

                if KSTAGE < 2:
                    for c0 in range(0, ncol, 512):
                        radial_chunk(c0)
                    continue
                # banks: A=[s0|v1x|v1y|v1z]  B=[v0x|s1x|s1y|v0y]  Cb=[s1z|v0z|-|-]
                agg = ps_agg.tile([128, 3, 512], f32)

                chunks = [(t0, min(t0 + CH, TW)) for t0 in range(0, TW, CH)]
                for (t0, t1) in chunks:
                    nt = t1 - t0
                    g0 = w * TW + t0
                    # radial chunk c covers exactly edge tiles [8c, 8c+8)
                    if t0 * 64 < ncol:
                        radial_chunk(t0 * 64)
                    # ---- loads for this chunk
                    ohw_t = epool.tile([128, CH, 4, WIN], bf, tag="ohw")
                    nc.sync.dma_start(ohw_t[:, :nt], OHW_d[:, g0:g0 + nt, :, :])

                    # ---- m = y[send] via SWDGE gather;  w = h3 @ w3 per tile
                    m_sb = mwpool.tile([128, CH, 512], bf, tag="m")
                    w_sb = mwpool.tile([128, CH, 512], bf, tag="w")
                    nidx = nt * 128
                    nc.gpsimd.dma_gather(
                        m_sb[:, 0:nt, :], y_d[:],
                        sendidx_s[:, g0 * 8:g0 * 8 + nidx // 16],
                        nidx, nidx, 4 * C,
                    )
                    for tl in range(nt):
                        t = t0 + tl
                        s = t % 2
                        P = t // 2
                        psw = ps_w.tile([128, 512], f32, tag="w")
                        nc.tensor.matmul(
                            psw[:], lhsT=h3[s * 64:(s + 1) * 64,
                                            P * 128:(P + 1) * 128],
                            rhs=w3d_s[s * 64:(s + 1) * 64, :],
                            start=True, stop=True)
                        nc.scalar.copy(w_sb[:, tl, :], psw[:])

                    if KSTAGE < 3:
                        continue
                    # ---- products (DVE slabs over the chunk)
                    # w_sb cols: [ws0 | ws1' | wv0 | wv1]; m_sb: [m0|m1x|m1y|m1z]
                    P1 = ppool.tile([128, CH, 512], bf, tag="P1")
                    P2 = ppool.tile([128, CH, 384], bf, tag="P2")
                    P3 = ppool.tile([128, CH, 256], bf, tag="P3")

                    def mslice(j):
                        return m_sb[:, 0:nt, j * 128:(j + 1) * 128]

                    def wslice(j):
                        return w_sb[:, 0:nt, j * 128:(j + 1) * 128]

                    # P1 = [m0*ws0 | m1x*wv1 | m1y*wv1 | m1z*wv1]
                    nc.vector.tensor_tensor(out=P1[:, 0:nt, 0:128],
                                            in0=mslice(0), in1=wslice(0), op=OP.mult)
                    for j in range(3):
                        nc.vector.tensor_tensor(
                            out=P1[:, 0:nt, (1 + j) * 128:(2 + j) * 128],
                            in0=mslice(1 + j), in1=wslice(3), op=OP.mult)
                    # P2 = [m1y*ws1' | m0*wv0 | m1x*ws1']
                    nc.vector.tensor_tensor(out=P2[:, 0:nt, 0:128],
                                            in0=mslice(2), in1=wslice(1), op=OP.mult)
                    nc.vector.tensor_tensor(out=P2[:, 0:nt, 128:256],
                                            in0=mslice(0), in1=wslice(2), op=OP.mult)
                    nc.vector.tensor_tensor(out=P2[:, 0:nt, 256:384],
                                            in0=mslice(1), in1=wslice(1), op=OP.mult)
                    # P3 = [m1z*ws1' | m0*wv0 (copy)]
                    nc.vector.tensor_tensor(out=P3[:, 0:nt, 0:128],
                                            in0=mslice(3), in1=wslice(1), op=OP.mult)
                    nc.vector.tensor_copy(P3[:, 0:nt, 128:256], P2[:, 0:nt, 128:256])

                    if KDEBUG and w == 0 and t0 == 0:
                        nc.sync.dma_start(dbgm_d[:], m_sb[:])
                        nc.sync.dma_start(dbgw_d[:], w_sb[:])
                        nc.sync.dma_start(dbg1_d[:], P1[:])
                        nc.sync.dma_start(dbg2_d[:], P2[:])
                        nc.sync.dma_start(dbg3_d[:], P3[:])

                    if KSTAGE < 4:
                        continue
                    # ---- scatter: A=[s0|v1x|v1y|v1z] B=[v0x|s1x|s1y|v0y]
                    #               Cb=[s1z|v0z]  (regions disjoint per tile)
                    for tl in range(nt):
                        t = t0 + tl
                        first = (t == 0)
                        last = (t == TW - 1)
                        oh = ohw_t[:, tl, 0, :]
                        ohx = ohw_t[:, tl, 1, :]
                        ohy = ohw_t[:, tl, 2, :]
                        ohz = ohw_t[:, tl, 3, :]
                        nc.tensor.matmul(agg[:, 0, :], lhsT=oh,
                                         rhs=P1[:, tl, 0:512],
                                         start=first, stop=last,
                                         skip_group_check=True)
                        # x: [v0x | s1x] <- ohx @ [m0wv0 | m1x*ws1']
                        # NOTE: start=True resets has_written for the WHOLE
                        # PSUM bank -> only the bank's first MM may set it.
                        nc.tensor.matmul(agg[:, 1, 0:256], lhsT=ohx,
                                         rhs=P2[:, tl, 128:384],
                                         start=first, stop=False,
                                         skip_group_check=True)
                        # y: [s1y | v0y] <- ohy @ [m1y*ws1' | m0wv0]
                        nc.tensor.matmul(agg[:, 1, 256:512], lhsT=ohy,
                                         rhs=P2[:, tl, 0:256],
                                         start=False, stop=last,
                                         skip_group_check=True)
                        # z: [s1z | v0z] <- ohz @ [m1z*ws1' | m0wv0]
                        nc.tensor.matmul(agg[:, 2, 0:256], lhsT=ohz,
                                         rhs=P3[:, tl, 0:256],
                                         start=first, stop=last,
                                         skip_group_check=True)

                # ================= final per window =================
                if KSTAGE < 5:
                    continue
                agg_sb = fpool.tile([128, 3, 512], bf, tag="aggsb")
                nc.scalar.copy(agg_sb[:, 0, :], agg[:, 0, :])
                nc.scalar.copy(agg_sb[:, 1, :], agg[:, 1, :])
                nc.scalar.copy(agg_sb[:, 2, 0:256], agg[:, 2, 0:256])

                if KDEBUG and w == 0:
                    nc.sync.dma_start(dbga_d[:], agg_sb[:])

                # blocks: 0=s0 1=v1x 2=v1y 3=v1z 4=v0x 5=s1x 6=s1y 7=v0y
                #         8=s1z 9=v0z
                aggT = fpool.tile([128, 10, 128], bf, tag="aggT")
                for b in range(10):
                    pst = ps_sm.tile([128, 128], bf, tag="sm")
                    nc.tensor.transpose(pst[:],
                                        agg_sb[:, b // 4, (b % 4) * 128:
                                               (b % 4 + 1) * 128],
                                        identity=ident_s[:])
                    nc.vector.tensor_copy(aggT[:, b, :], pst[:])

                pss = ps_sm.tile([128, 2, 128], f32, tag="sm")
                for mch in range(2):
                    nc.tensor.matmul(
                        pss[:, mch, :],
                        lhsT=W20_s[:, 0, mch * 128:(mch + 1) * 128],
                        rhs=aggT[:, 0, :], start=True, stop=False,
                        skip_group_check=True)
                    for s1b in (5, 6, 8):
                        nc.tensor.matmul(
                            pss[:, mch, :],
                            lhsT=W20_s[:, 1, mch * 128:(mch + 1) * 128],
                            rhs=aggT[:, s1b, :], start=False, stop=False,
                            skip_group_check=True)
                    for k in range(NS):
                        nc.tensor.matmul(
                            pss[:, mch, :],
                            lhsT=Wsk0_s[:, k, mch * 128:(mch + 1) * 128],
                            rhs=xskT_s[:, k, w * 128:(w + 1) * 128],
                            start=False, stop=(k == NS - 1),
                            skip_group_check=True)
                outs = fpool.tile([128, 128], bf, tag="outs")
                nc.scalar.activation(outs[:], pss[:, 0, :], AF.Silu)
                gates = fpool.tile([128, 128], bf, tag="gates")
                nc.scalar.activation(gates[:], pss[:, 1, :], AF.Silu)

                v0blk = [4, 7, 9]
                psv = ps_sm.tile([128, 3, 128], f32, tag="sm")
                for i in range(3):
                    nc.tensor.matmul(psv[:, i, :], lhsT=W21_s[:, 0, :],
                                     rhs=aggT[:, v0blk[i], :],
                                     start=True, stop=False,
                                     skip_group_check=True)
                    nc.tensor.matmul(psv[:, i, :], lhsT=W21_s[:, 1, :],
                                     rhs=aggT[:, 1 + i, :],
                                     start=False, stop=False,
                                     skip_group_check=True)
                    for k in range(NS):
                        nc.tensor.matmul(
                            psv[:, i, :], lhsT=Wsk1_s[:, k, :],
                            rhs=xvkT_s[:, k, i, w * 128:(w + 1) * 128],
                            start=False, stop=(k == NS - 1),
                            skip_group_check=True)

                ow = fpool.tile([128, 4, 128], f32, tag="ow")
                nc.vector.tensor_copy(ow[:, 0, :], outs[:])
                for i in range(3):
                    nc.vector.tensor_tensor(out=ow[:, 1 + i, :], in0=psv[:, i, :],
                                            in1=gates[:], op=OP.mult)
                for fch in range(4):
                    nc.sync.dma_start(
                        outT_d[fch * 128:(fch + 1) * 128, w * 128:(w + 1) * 128],
                        ow[:, fch, :])
    nc.compile()
    return nc


_CACHE = {}


def kernel(**inputs):
    from concourse.bass_utils import run_bass_kernel_spmd
    consts, cores, EW = _prep_host(inputs)
    if EW not in _CACHE:
        _CACHE[EW] = _build_program(EW)
    nc = _CACHE[EW]
    in_maps = []
    for i in range(NCORES):
        m = dict(consts)
        m.update(cores[i])
        in_maps.append(m)
    res = run_bass_kernel_spmd(nc, in_maps, list(range(NCORES)))
    out = np.zeros((NCORES, NV, 4 * C), np.float32)
    for i in range(NCORES):
        outT = np.asarray(res.results[i]["outT"], np.float32)   # [512, 512]
        full = outT[:, :NV].T                                   # [NV, 512]
        out_s = full[:, 0:C]
        v = np.stack([full[:, C:2 * C], full[:, 2 * C:3 * C], full[:, 3 * C:]],
                     axis=2).reshape(NV, 3 * C)
        out[i] = np.concatenate([out_s, v], axis=1)
    return out.reshape(N, 4 * C).astype(np.float32)


# revision 42
# speedup vs baseline: 1.1362x; 1.1362x over previous
"""Trainium2 Bass kernel for nn_Nequix (e3nn-style message-passing layer).

Sharding: edges partitioned by receiver range across 8 cores (500 nodes each);
node features and weights replicated; no collectives. Within a core, edges are
grouped into 4 windows of 128 receiver nodes; scatter-sum is one-hot matmuls
accumulating in PSUM per window.

v2 layout (all per core):
  xgT  [C, TT, 4, 128]  pre-gathered sender features (host gather), per-tile
                        lhsT for m = x[send] @ W1 on the PE (no device gather)
  OHW  [128, TT, 4, WIN] one-hot + Y1-weighted one-hots (host); scatter lhsT
  rb2  [16, EPAD/2]     radial basis, 2 edge-tiles packed per column pair-slot
  radial MLP            block-diagonal 2-slot weights -> h slabs [128, EW/2]
  L3                    per-tile h3 lhsT -> w [128e, 4C] PSUM, ACT evac
  products (DVE)        9 slab mults m (.) w per 8-tile chunk
  scatter               agg PSUM [128, 2, 512]: A=[s0|v1x|v1y|v1z],
                        B=[v0x|s1|v0y|v0z]; lhsT = OHW slices
  final                 transpose agg, linear_2 + species skip, silu gates;
                        outT [512, 512] f32
"""
import math
import os
import numpy as np

KSTAGE = int(os.environ.get("KSTAGE", "9"))
KDEBUG = int(os.environ.get("KDEBUG", "0"))

N, E, C, NS, RB, H = 4000, 128000, 128, 5, 8, 64
AVG_N = 32.0
NCORES = 8
NV = 500
WIN = 128
NWIN = 4
SQ3 = math.sqrt(3.0)


def _prep_host(inputs):
    import ml_dtypes
    bf = ml_dtypes.bfloat16
    f32 = np.float32

    xs = np.asarray(inputs["x_scalars"], f32)
    xv = np.asarray(inputs["x_vectors"], f32)
    ev = np.asarray(inputs["edge_vectors"], f32)
    rb = np.asarray(inputs["radial_basis"], f32)
    W1_0 = np.asarray(inputs["W1_0"], f32)
    W1_1 = np.asarray(inputs["W1_1"], f32)
    w0 = np.asarray(inputs["rmlp_w0"], f32)
    w1 = np.asarray(inputs["rmlp_w1"], f32)
    w2 = np.asarray(inputs["rmlp_w2"], f32)
    w3 = np.asarray(inputs["rmlp_w3"], f32).copy()
    W2_0 = np.asarray(inputs["W2_0"], f32)
    W2_1 = np.asarray(inputs["W2_1"], f32)
    Wsk0 = np.asarray(inputs["Wsk0"], f32)
    Wsk1 = np.asarray(inputs["Wsk1"], f32)
    species = np.asarray(inputs["species"]).astype(np.int64)
    send = np.asarray(inputs["senders"]).astype(np.int64)
    recv = np.asarray(inputs["receivers"]).astype(np.int64)

    inv_c = f32(1.0 / math.sqrt(C))
    W1_0f = W1_0 * inv_c
    W1_1f = W1_1 * inv_c
    w3f = w3 * f32(1.0 / math.sqrt(AVG_N))
    w3f[:, C:2 * C] *= f32(1.0 / SQ3)
    inv_2c = f32(1.0 / math.sqrt(2 * C))
    W2_0f = W2_0 * inv_2c
    W2_1f = W2_1 * inv_2c
    Wsk0f = Wsk0 * inv_c          # [NS, C, 2C]
    Wsk1f = Wsk1 * inv_c          # [NS, C, C]

    # edge geometry (host): Y1 = sqrt(3) * unit(edge_vectors)
    r = np.sqrt((ev * ev).sum(1, keepdims=True))
    Y1 = SQ3 * ev / np.maximum(r, 1e-12)                               # [E,3]

    # ---- edge shard assignment: core by receiver range, window by rloc//WIN
    core_of = recv // NV
    rloc_all = recv - core_of * NV
    win_of = rloc_all // WIN
    per_core_win = []
    maxcnt = 0
    for i in range(NCORES):
        wins = []
        for w in range(NWIN):
            pw = np.nonzero((core_of == i) & (win_of == w))[0]
            wins.append(pw)
            maxcnt = max(maxcnt, len(pw))
        per_core_win.append(wins)
    EW = ((maxcnt + 255) // 256) * 256     # even tile count per window
    TW = EW // 128
    EPAD = EW * NWIN
    TT = EPAD // 128

    # ---- shared constants
    # block-diagonal 2-slot radial weights
    w0b = np.zeros((16, 128), f32)
    w0b[:RB, :H] = w0
    w0b[RB:2 * RB, H:2 * H] = w0
    w1b = np.zeros((128, 128), f32)
    w1b[:H, :H] = w1
    w1b[H:, H:] = w1
    w2b = np.zeros((128, 128), f32)
    w2b[:H, :H] = w2
    w2b[H:, H:] = w2
    w3d = np.concatenate([w3f, w3f], axis=0)                           # [128,4C]

    W20L = np.stack([W2_0f[0:128, :], W2_0f[128:256, :]], axis=1)      # [128,2,256]
    W21L = np.stack([W2_1f[0:128, :], W2_1f[128:256, :]], axis=1)      # [128,2,128]
    Wsk0L = Wsk0f.transpose(1, 0, 2)                                   # [128,NS,256]
    Wsk1L = Wsk1f.transpose(1, 0, 2)                                   # [128,NS,128]

    NPAD = 4096
    xsT = np.zeros((C, NPAD), f32)
    xsT[:, :N] = xs.T
    xvT = np.zeros((3, C, NPAD), f32)
    for i in range(3):
        xvT[i, :, :N] = xv[:, :, i].T

    consts = dict(
        xsT=xsT.astype(bf), xvT=xvT.astype(bf),
        W10=W1_0f.astype(bf), W11=W1_1f.astype(bf),
        w0b=w0b.astype(bf), w1b=w1b.astype(bf), w2b=w2b.astype(bf),
        w3d=w3d.astype(bf),
        W20=W20L.astype(bf), W21=W21L.astype(bf),
        Wsk0=Wsk0L.astype(bf), Wsk1=Wsk1L.astype(bf),
    )

    # ---- per-core tensors
    cores = []
    for i in range(NCORES):
        send_p = np.zeros(EPAD, np.int64)
        rloc_p = np.zeros(EPAD, np.int64)
        val_p = np.zeros(EPAD, f32)
        rb_p = np.zeros((EPAD, RB), f32)
        Y1_p = np.zeros((EPAD, 3), f32)
        for w in range(NWIN):
            pw = per_core_win[i][w]
            k = len(pw)
            sl = slice(w * EW, w * EW + k)
            send_p[sl] = send[pw]
            rloc_p[sl] = rloc_all[pw] - w * WIN
            val_p[sl] = 1.0
            rb_p[sl] = rb[pw]
            Y1_p[sl] = Y1[pw]

        # gather indices, int16 (pad slots gather row 0; one-hot row is zero)
        idx16 = send_p.astype(np.int16).reshape(EPAD // 16, 16).T
        sendidx = np.tile(idx16, (8, 1))                               # [128, EPAD//16]

        # OHW[p, t, 0, n] = onehot; [p, t, 1+i, n] = onehot * Y1_i
        ohw = np.zeros((EPAD, 4, WIN), f32)
        ar = np.arange(EPAD)
        ohw[ar, 0, rloc_p] = val_p
        for j in range(3):
            ohw[ar, 1 + j, rloc_p] = val_p * Y1_p[:, j]
        OHW = ohw.reshape(TT, 128, 4, WIN).transpose(1, 0, 2, 3)

        # rb2[s*8+r, w*(EW/2) + P*128 + p] = rb[edge (w, (2P+s)*128+p), r]
        arr = rb_p.reshape(NWIN, TW // 2, 2, 128, RB)
        rb2 = arr.transpose(2, 4, 0, 1, 3).reshape(16, EPAD // 2)

        sl_n = slice(i * NV, (i + 1) * NV)
        xs_my = np.zeros((512, C), f32)
        xs_my[:NV] = xs[sl_n]
        xv_my = np.zeros((512, C, 3), f32)
        xv_my[:NV] = xv[sl_n]
        soh = np.zeros((512, NS), f32)
        soh[np.arange(NV), species[sl_n]] = 1.0
        xskT = np.einsum("nc,nk->ckn", xs_my, soh)                     # [128,NS,512]
        xvkT = np.einsum("nci,nk->ckin", xv_my, soh)                   # [128,NS,3,512]

        cores.append(dict(
            sendidx=np.ascontiguousarray(sendidx),
            OHW=np.ascontiguousarray(OHW.astype(bf)),
            rb2=np.ascontiguousarray(rb2.astype(bf)),
            xskT=np.ascontiguousarray(xskT.astype(bf)),
            xvkT=np.ascontiguousarray(xvkT.astype(bf)),
        ))
    return consts, cores, EW


def _build_program(EW):
    import concourse.bacc as bacc
    import concourse.mybir as mybir
    import concourse.tile as tile
    from concourse.masks import make_identity

    f32 = mybir.dt.float32
    bf = mybir.dt.bfloat16
    AF = mybir.ActivationFunctionType
    OP = mybir.AluOpType

    TW = EW // 128
    EPAD = EW * NWIN
    TT = EPAD // 128
    CH = 6                                  # edge tiles per compute chunk

    nc = bacc.Bacc("TRN2", target_bir_lowering=False)

    def param(name, shape, dtype):
        return nc.declare_dram_parameter(name, list(shape), dtype, isOutput=False)

    NPAD = 4096
    i16 = mybir.dt.int16
    xsT_d = param("xsT", (C, NPAD), bf)
    xvT_d = param("xvT", (3, C, NPAD), bf)
    sendidx_d = param("sendidx", (128, EPAD // 16), i16)
    W10_d = param("W10", (C, C), bf)
    W11_d = param("W11", (C, C), bf)
    w0b_d = param("w0b", (16, 128), bf)
    w1b_d = param("w1b", (128, 128), bf)
    w2b_d = param("w2b", (128, 128), bf)
    w3d_d = param("w3d", (128, 4 * C), bf)
    W20_d = param("W20", (C, 2, 2 * C), bf)
    W21_d = param("W21", (C, 2, C), bf)
    Wsk0_d = param("Wsk0", (C, NS, 2 * C), bf)
    Wsk1_d = param("Wsk1", (C, NS, C), bf)
    OHW_d = param("OHW", (128, TT, 4, WIN), bf)
    rb2_d = param("rb2", (16, EPAD // 2), bf)
    xskT_d = param("xskT", (C, NS, 512), bf)
    xvkT_d = param("xvkT", (C, NS, 3, 512), bf)
    outT_d = nc.declare_dram_parameter("outT", [4 * C, 512], f32, isOutput=True)
    if KDEBUG:
        dbgm_d = nc.declare_dram_parameter("dbgm", [128, CH, 512], bf, isOutput=True)
        dbgw_d = nc.declare_dram_parameter("dbgw", [128, CH, 512], bf, isOutput=True)
        dbg1_d = nc.declare_dram_parameter("dbg1", [128, CH, 512], bf, isOutput=True)
        dbg2_d = nc.declare_dram_parameter("dbg2", [128, CH, 384], bf, isOutput=True)
        dbg3_d = nc.declare_dram_parameter("dbg3", [128, CH, 256], bf, isOutput=True)
        dbga_d = nc.declare_dram_parameter("dbga", [128, 3, 512], bf, isOutput=True)

    with tile.TileContext(nc) as tc:
        with (
            tc.tile_pool(name="dram", bufs=1, space="DRAM") as dpool,
            tc.tile_pool(name="const", bufs=1) as cpool,
            tc.tile_pool(name="xload", bufs=1) as xpool,
            tc.tile_pool(name="rbload", bufs=2) as rbpool,
            tc.tile_pool(name="hslab", bufs=2) as hpool,
            tc.tile_pool(name="eload", bufs=2) as epool,
            tc.tile_pool(name="mw", bufs=2) as mwpool,
            tc.tile_pool(name="prod", bufs=2) as ppool,
            tc.tile_pool(name="fin", bufs=2) as fpool,
            tc.tile_pool(name="ps_w", bufs=3, space="PSUM") as ps_w,
            tc.tile_pool(name="ps_sm", bufs=1, space="PSUM") as ps_sm,
            tc.tile_pool(name="ps_agg", bufs=1, space="PSUM") as ps_agg,
        ):
            y_d = dpool.tile([NPAD, 4 * C], bf)
            def cload(dram, shape, dtype):
                t = cpool.tile(list(shape), dtype, tag=dram.name)
                nc.sync.dma_start(t[:], dram[:])
                return t

            W10_s = cload(W10_d, (C, C), bf)
            W11_s = cload(W11_d, (C, C), bf)
            w0b_s = cload(w0b_d, (16, 128), bf)
            w1b_s = cload(w1b_d, (128, 128), bf)
            w2b_s = cload(w2b_d, (128, 128), bf)
            w3d_s = cload(w3d_d, (128, 4 * C), bf)
            W20_s = cload(W20_d, (C, 2, 2 * C), bf)
            W21_s = cload(W21_d, (C, 2, C), bf)
            Wsk0_s = cload(Wsk0_d, (C, NS, 2 * C), bf)
            Wsk1_s = cload(Wsk1_d, (C, NS, C), bf)
            xskT_s = cload(xskT_d, (C, NS, 512), bf)
            xvkT_s = cload(xvkT_d, (C, NS, 3, 512), bf)
            sendidx_s = cload(sendidx_d, (128, EPAD // 16), i16)
            ident_s = cpool.tile([128, 128], bf)
            make_identity(nc, ident_s[:])

            # ================= phase Y: y = x @ W1 (all nodes) =================
            # slab-load x (4 dma_starts per tensor half) then stream matmuls
            for half in range(2):
                h0 = half * 2048
                xs_sl = xpool.tile([C, 2048], bf, tag="xs")
                for q in range(2):
                    nc.sync.dma_start(xs_sl[:, q * 1024:(q + 1) * 1024],
                                      xsT_d[:, h0 + q * 1024:h0 + (q + 1) * 1024])
                xv_sl = xpool.tile([C, 3, 2048], bf, tag="xv")
                for i in range(3):
                    nc.sync.dma_start(xv_sl[:, i, :], xvT_d[i, :, h0:h0 + 2048])
                for nch in range(16):
                    csl = slice(nch * 128, (nch + 1) * 128)
                    psy = ps_w.tile([128, 4 * C], f32, tag="w")
                    nc.tensor.matmul(psy[:, 0:C], lhsT=xs_sl[:, csl], rhs=W10_s[:],
                                     start=True, stop=True)
                    for i in range(3):
                        nc.tensor.matmul(psy[:, (1 + i) * C:(2 + i) * C],
                                         lhsT=xv_sl[:, i, csl],
                                         rhs=W11_s[:], start=True, stop=True)
                    yb = xpool.tile([128, 4 * C], bf, tag="yb", bufs=2)
                    nc.scalar.copy(yb[:], psy[:])
                    nc.sync.dma_start(y_d[h0 + nch * 128:h0 + (nch + 1) * 128, :],
                                      yb[:])

            for w in range(NWIN if KSTAGE >= 1 else 0):
                # ---------------- radial MLP (2-slot packed) ----------------
                ncol = EW // 2
                rb2_t = rbpool.tile([16, ncol], bf, tag="rb2")
                nc.sync.dma_start(rb2_t[:], rb2_d[:, w * ncol:(w + 1) * ncol])
                h3 = hpool.tile([128, ncol], bf, tag="h3")
                h1 = hpool.tile([128, 512], bf, tag="h1")
                h2 = hpool.tile([128, 512], bf, tag="h2")

                def radial_chunk(c0, nn):
                    nn = min(nn, ncol - c0)
                    c1 = c0 + nn
                    ps0 = ps_sm.tile([128, 512], f32, tag="sm", name="ps0")
                    nc.tensor.matmul(ps0[:, :nn], lhsT=w0b_s[:],
                                     rhs=rb2_t[:, c0:c1], start=True, stop=True)
                    nc.scalar.activation(h1[:, :nn], ps0[:, :nn], AF.Silu)
                    ps1 = ps_sm.tile([128, 512], f32, tag="sm", name="ps1")
                    nc.tensor.matmul(ps1[:, :nn], lhsT=w1b_s[:], rhs=h1[:, :nn],
                                     start=True, stop=True)
                    nc.scalar.activation(h2[:, :nn], ps1[:, :nn], AF.Silu)
                    ps2 = ps_sm.tile([128, 512], f32, tag="sm", name="ps2")
                    nc.tensor.matmul(ps2[:, :nn], lhsT=w2b_s[:], rhs=h2[:, :nn],
                                     start=True, stop=True)
                    nc.scalar.activation(h3[:, c0:c1], ps2[:, :nn], AF.Silu)

                if KSTAGE < 2:
                    for c0 in range(0, ncol, 512):
                        radial_chunk(c0, 512)
                    continue
                # banks: A=[s0|v1x|v1y|v1z]  B=[v0x|s1x|s1y|v0y]  Cb=[s1z|v0z|-|-]
                agg = ps_agg.tile([128, 3, 512], f32)

                chunks = [(t0, min(t0 + CH, TW)) for t0 in range(0, TW, CH)]
                for (t0, t1) in chunks:
                    nt = t1 - t0
                    g0 = w * TW + t0
                    # radial cols [t0*64, t1*64) feed exactly edge tiles [t0, t1)
                    radial_chunk(t0 * 64, nt * 64)
                    # ---- loads for this chunk
                    ohw_t = epool.tile([128, CH, 4, WIN], bf, tag="ohw")
                    nc.sync.dma_start(ohw_t[:, :nt], OHW_d[:, g0:g0 + nt, :, :])

                    # ---- m = y[send] via SWDGE gather;  w = h3 @ w3 per tile
                    m_sb = mwpool.tile([128, CH, 512], bf, tag="m")
                    w_sb = mwpool.tile([128, CH, 512], bf, tag="w")
                    nidx = nt * 128
                    nc.gpsimd.dma_gather(
                        m_sb[:, 0:nt, :], y_d[:],
                        sendidx_s[:, g0 * 8:g0 * 8 + nidx // 16],
                        nidx, nidx, 4 * C,
                    )
                    for tl in range(nt):
                        t = t0 + tl
                        s = t % 2
                        P = t // 2
                        psw = ps_w.tile([128, 512], f32, tag="w")
                        nc.tensor.matmul(
                            psw[:], lhsT=h3[s * 64:(s + 1) * 64,
                                            P * 128:(P + 1) * 128],
                            rhs=w3d_s[s * 64:(s + 1) * 64, :],
                            start=True, stop=True)
                        nc.scalar.copy(w_sb[:, tl, :], psw[:])

                    if KSTAGE < 3:
                        continue
                    # ---- products (DVE slabs over the chunk)
                    # w_sb cols: [ws0 | ws1' | wv0 | wv1]; m_sb: [m0|m1x|m1y|m1z]
                    P1 = ppool.tile([128, CH, 512], bf, tag="P1")
                    P2 = ppool.tile([128, CH, 384], bf, tag="P2")
                    P3 = ppool.tile([128, CH, 256], bf, tag="P3")

                    def mslice(j):
                        return m_sb[:, 0:nt, j * 128:(j + 1) * 128]

                    def wslice(j):
                        return w_sb[:, 0:nt, j * 128:(j + 1) * 128]

                    # P1 = [m0*ws0 | m1x*wv1 | m1y*wv1 | m1z*wv1]
                    nc.vector.tensor_tensor(out=P1[:, 0:nt, 0:128],
                                            in0=mslice(0), in1=wslice(0), op=OP.mult)
                    for j in range(3):
                        nc.vector.tensor_tensor(
                            out=P1[:, 0:nt, (1 + j) * 128:(2 + j) * 128],
                            in0=mslice(1 + j), in1=wslice(3), op=OP.mult)
                    # P2 = [m1y*ws1' | m0*wv0 | m1x*ws1']
                    nc.vector.tensor_tensor(out=P2[:, 0:nt, 0:128],
                                            in0=mslice(2), in1=wslice(1), op=OP.mult)
                    nc.vector.tensor_tensor(out=P2[:, 0:nt, 128:256],
                                            in0=mslice(0), in1=wslice(2), op=OP.mult)
                    nc.vector.tensor_tensor(out=P2[:, 0:nt, 256:384],
                                            in0=mslice(1), in1=wslice(1), op=OP.mult)
                    # P3 = [m1z*ws1' | m0*wv0 (copy)]
                    nc.vector.tensor_tensor(out=P3[:, 0:nt, 0:128],
                                            in0=mslice(3), in1=wslice(1), op=OP.mult)
                    nc.vector.tensor_copy(P3[:, 0:nt, 128:256], P2[:, 0:nt, 128:256])

                    if KDEBUG and w == 0 and t0 == 0:
                        nc.sync.dma_start(dbgm_d[:], m_sb[:])
                        nc.sync.dma_start(dbgw_d[:], w_sb[:])
                        nc.sync.dma_start(dbg1_d[:], P1[:])
                        nc.sync.dma_start(dbg2_d[:], P2[:])
                        nc.sync.dma_start(dbg3_d[:], P3[:])

                    if KSTAGE < 4:
                        continue
                    # ---- scatter: A=[s0|v1x|v1y|v1z] B=[v0x|s1x|s1y|v0y]
                    #               Cb=[s1z|v0z]  (regions disjoint per tile)
                    for tl in range(nt):
                        t = t0 + tl
                        first = (t == 0)
                        last = (t == TW - 1)
                        oh = ohw_t[:, tl, 0, :]
                        ohx = ohw_t[:, tl, 1, :]
                        ohy = ohw_t[:, tl, 2, :]
                        ohz = ohw_t[:, tl, 3, :]
                        nc.tensor.matmul(agg[:, 0, :], lhsT=oh,
                                         rhs=P1[:, tl, 0:512],
                                         start=first, stop=last,
                                         skip_group_check=True)
                        # x: [v0x | s1x] <- ohx @ [m0wv0 | m1x*ws1']
                        # NOTE: start=True resets has_written for the WHOLE
                        # PSUM bank -> only the bank's first MM may set it.
                        nc.tensor.matmul(agg[:, 1, 0:256], lhsT=ohx,
                                         rhs=P2[:, tl, 128:384],
                                         start=first, stop=False,
                                         skip_group_check=True)
                        # y: [s1y | v0y] <- ohy @ [m1y*ws1' | m0wv0]
                        nc.tensor.matmul(agg[:, 1, 256:512], lhsT=ohy,
                                         rhs=P2[:, tl, 0:256],
                                         start=False, stop=last,
                                         skip_group_check=True)
                        # z: [s1z | v0z] <- ohz @ [m1z*ws1' | m0wv0]
                        nc.tensor.matmul(agg[:, 2, 0:256], lhsT=ohz,
                                         rhs=P3[:, tl, 0:256],
                                         start=first, stop=last,
                                         skip_group_check=True)

                # ================= final per window =================
                if KSTAGE < 5:
                    continue
                agg_sb = fpool.tile([128, 3, 512], bf, tag="aggsb")
                nc.scalar.copy(agg_sb[:, 0, :], agg[:, 0, :])
                nc.scalar.copy(agg_sb[:, 1, :], agg[:, 1, :])
                nc.scalar.copy(agg_sb[:, 2, 0:256], agg[:, 2, 0:256])

                if KDEBUG and w == 0:
                    nc.sync.dma_start(dbga_d[:], agg_sb[:])

                # blocks: 0=s0 1=v1x 2=v1y 3=v1z 4=v0x 5=s1x 6=s1y 7=v0y
                #         8=s1z 9=v0z
                aggT = fpool.tile([128, 10, 128], bf, tag="aggT")
                for b in range(10):
                    pst = ps_sm.tile([128, 128], bf, tag="sm")
                    nc.tensor.transpose(pst[:],
                                        agg_sb[:, b // 4, (b % 4) * 128:
                                               (b % 4 + 1) * 128],
                                        identity=ident_s[:])
                    nc.vector.tensor_copy(aggT[:, b, :], pst[:])

                pss = ps_sm.tile([128, 2, 128], f32, tag="sm")
                for mch in range(2):
                    nc.tensor.matmul(
                        pss[:, mch, :],
                        lhsT=W20_s[:, 0, mch * 128:(mch + 1) * 128],
                        rhs=aggT[:, 0, :], start=True, stop=False,
                        skip_group_check=True)
                    for s1b in (5, 6, 8):
                        nc.tensor.matmul(
                            pss[:, mch, :],
                            lhsT=W20_s[:, 1, mch * 128:(mch + 1) * 128],
                            rhs=aggT[:, s1b, :], start=False, stop=False,
                            skip_group_check=True)
                    for k in range(NS):
                        nc.tensor.matmul(
                            pss[:, mch, :],
                            lhsT=Wsk0_s[:, k, mch * 128:(mch + 1) * 128],
                            rhs=xskT_s[:, k, w * 128:(w + 1) * 128],
                            start=False, stop=(k == NS - 1),
                            skip_group_check=True)
                outs = fpool.tile([128, 128], bf, tag="outs")
                nc.scalar.activation(outs[:], pss[:, 0, :], AF.Silu)
                gates = fpool.tile([128, 128], bf, tag="gates")
                nc.scalar.activation(gates[:], pss[:, 1, :], AF.Silu)

                v0blk = [4, 7, 9]
                psv = ps_sm.tile([128, 3, 128], f32, tag="sm")
                for i in range(3):
                    nc.tensor.matmul(psv[:, i, :], lhsT=W21_s[:, 0, :],
                                     rhs=aggT[:, v0blk[i], :],
                                     start=True, stop=False,
                                     skip_group_check=True)
                    nc.tensor.matmul(psv[:, i, :], lhsT=W21_s[:, 1, :],
                                     rhs=aggT[:, 1 + i, :],
                                     start=False, stop=False,
                                     skip_group_check=True)
                    for k in range(NS):
                        nc.tensor.matmul(
                            psv[:, i, :], lhsT=Wsk1_s[:, k, :],
                            rhs=xvkT_s[:, k, i, w * 128:(w + 1) * 128],
                            start=False, stop=(k == NS - 1),
                            skip_group_check=True)

                ow = fpool.tile([128, 4, 128], f32, tag="ow")
                nc.vector.tensor_copy(ow[:, 0, :], outs[:])
                for i in range(3):
                    nc.vector.tensor_tensor(out=ow[:, 1 + i, :], in0=psv[:, i, :],
                                            in1=gates[:], op=OP.mult)
                for fch in range(4):
                    nc.sync.dma_start(
                        outT_d[fch * 128:(fch + 1) * 128, w * 128:(w + 1) * 128],
                        ow[:, fch, :])
    nc.compile()
    return nc


_CACHE = {}


def kernel(**inputs):
    from concourse.bass_utils import run_bass_kernel_spmd
    consts, cores, EW = _prep_host(inputs)
    if EW not in _CACHE:
        _CACHE[EW] = _build_program(EW)
    nc = _CACHE[EW]
    in_maps = []
    for i in range(NCORES):
        m = dict(consts)
        m.update(cores[i])
        in_maps.append(m)
    res = run_bass_kernel_spmd(nc, in_maps, list(range(NCORES)))
    out = np.zeros((NCORES, NV, 4 * C), np.float32)
    for i in range(NCORES):
        outT = np.asarray(res.results[i]["outT"], np.float32)   # [512, 512]
        full = outT[:, :NV].T                                   # [NV, 512]
        out_s = full[:, 0:C]
        v = np.stack([full[:, C:2 * C], full[:, 2 * C:3 * C], full[:, 3 * C:]],
                     axis=2).reshape(NV, 3 * C)
        out[i] = np.concatenate([out_s, v], axis=1)
    return out.reshape(N, 4 * C).astype(np.float32)
